# revision 13
# baseline (speedup 1.0000x reference)
"""Multi-head attention (B=2, S=4096, D=512, H=8, HD=64, fp32) on 8 TRN2 cores.

Sharding: core c -> batch b = c//4, local head pair hp = c%4
(global heads 2*hp, 2*hp+1).  Attention is head-independent, so there is no
cross-core communication: each core computes a full softmax-attention for its
two heads over the whole sequence and writes its [S, 128] column slice of the
output.

Per-core dataflow (everything "transposed" so the contraction dim is always
the partition dim and no on-chip transposes of big tensors are needed):

  inputs (host-prepared):
    xT   [512, 4096] bf16   x[b].T
    wqT  [512, 128]  bf16   Wq[rows].T * 0.125 (score scale folded in)
    wkT, wvT              likewise (unscaled)
    bq   [128, 1] f32 (pre-scaled), bk [128, 1] f32
    bvb  [128, 128] f32     bv replicated across partitions
    ident [128, 128] f32    identity for PE transposes

  QT = wqT.T @ xT -> [128(2 heads x 64), S] bf16   (bias via DVE tensor_scalar)
  KT likewise.
  V  = x @ Wv.T   -> [S, 128] computed directly in [s, o] layout
       (xT chunks as the stationary operand); stored per head as
       [128, 65]-tiles with a constant 1.0 column appended (col 64) so the
       attn@V matmul also produces the softmax row-sums.

  Emission order: K proj, Q proj block 0, V proj, then the attention loop
  (Q proj block qg+1 emitted inside iteration qg) so ScalarE starts exp'ing
  as early as possible -- the kernel is ScalarE(exp)-bound.

  per q-group of 512 queries, per 128-wide key chunk kc:
    one PSUM tile [128, 1024] holds BOTH heads' scoresT chunks (bank-aligned
    halves); the two score matmuls (lhsT = KT chunk [64, 128]) use PE rows
    0-63 / 64-127 and run concurrently (row-tiled).  One ScalarE exp instr
    [128, 1024] (PSUM->SBUF bf16; no max subtraction: |scores| <~ 3, exp
    cannot overflow).  attn@V accumulates outT_plus [65, 512q] per head over
    the 32 k-chunks (row 64 = softmax denominator).  Tail: PE-transpose
    [65, 128] blocks to [128, 65], multiply by reciprocal(denominator), DMA
    the [128, 128] f32 out tile.
"""

import numpy as np

B, S, D, H = 2, 4096, 512, 8
HD = D // H          # 64
OD = 128             # output dims per core (2 heads)
QW = 512             # query group width

_CACHE = {}


def _build(s=S, rep=1, ufd=2):
    import concourse.bacc as bacc
    import concourse.mybir as mybir
    import concourse.tile as tile

    f32 = mybir.dt.float32
    bf16 = mybir.dt.bfloat16
    Exp = mybir.ActivationFunctionType.Exp

    qg_n = s // QW
    kc_n = s // 128
    sb_n = s // QW

    nc = bacc.Bacc(None, target_bir_lowering=False)

    xT = nc.dram_tensor("xT", [D, s], bf16, kind="ExternalInput")
    wqT = nc.dram_tensor("wqT", [D, OD], bf16, kind="ExternalInput")
    wkT = nc.dram_tensor("wkT", [D, OD], bf16, kind="ExternalInput")
    wvT = nc.dram_tensor("wvT", [D, OD], bf16, kind="ExternalInput")
    bq = nc.dram_tensor("bq", [OD, 1], f32, kind="ExternalInput")
    bk = nc.dram_tensor("bk", [OD, 1], f32, kind="ExternalInput")
    bvb = nc.dram_tensor("bvb", [128, OD], f32, kind="ExternalInput")
    ident = nc.dram_tensor("ident", [128, 128], f32, kind="ExternalInput")
    out = nc.dram_tensor("out", [s, OD], f32, kind="ExternalOutput")

    with tile.TileContext(nc) as tc:
        with (
            tc.tile_pool(name="persist", bufs=1) as persist,
            tc.tile_pool(name="exps", bufs=4) as exps,
            tc.tile_pool(name="outsb", bufs=8) as outsb,
            tc.tile_pool(name="outt", bufs=2) as outtp,
            tc.tile_pool(name="psbig", bufs=(3 if ufd == 2 else 2),
                         space="PSUM") as psbig,
            tc.tile_pool(name="pssmall", bufs=2, space="PSUM") as pssmall,
        ):
            # ---- constants / inputs to SBUF ----
            xt = [persist.tile([128, s], bf16, name=f"xt{c}", tag=f"xt{c}")
                  for c in range(4)]
            h2 = s // 2
            for c in range(4):
                nc.sync.dma_start(xt[c][:, 0:h2], xT[c * 128:(c + 1) * 128, 0:h2])
            for c in range(4):
                nc.sync.dma_start(xt[c][:, h2:s], xT[c * 128:(c + 1) * 128, h2:s])
            wq = [persist.tile([128, OD], bf16, name=f"wq{c}", tag=f"wq{c}")
                  for c in range(4)]
            wk = [persist.tile([128, OD], bf16, name=f"wk{c}", tag=f"wk{c}")
                  for c in range(4)]
            wv = [persist.tile([128, OD], bf16, name=f"wv{c}", tag=f"wv{c}")
                  for c in range(4)]
            for c in range(4):
                nc.sync.dma_start(wk[c][:], wkT[c * 128:(c + 1) * 128, :])
                nc.sync.dma_start(wq[c][:], wqT[c * 128:(c + 1) * 128, :])
                nc.sync.dma_start(wv[c][:], wvT[c * 128:(c + 1) * 128, :])
            bq_t = persist.tile([OD, 1], f32, name="bq_t", tag="bq")
            bk_t = persist.tile([OD, 1], f32, name="bk_t", tag="bk")
            bvb_t = persist.tile([128, OD], f32, name="bvb_t", tag="bvb")
            id_t = persist.tile([128, 128], f32, name="id_t", tag="ident")
            nc.sync.dma_start(bk_t[:], bk[:])
            nc.sync.dma_start(bq_t[:], bq[:])
            nc.sync.dma_start(bvb_t[:], bvb[:])
            nc.sync.dma_start(id_t[:], ident[:])

            qt = persist.tile([128, s], bf16, name="qt", tag="qt")
            kt = persist.tile([128, s], bf16, name="kt", tag="kt")

            def body():
                _emit_body(nc, tc, mybir, s, qt, kt, xt, wq, wk, wv,
                           bq_t, bk_t, bvb_t, id_t, out,
                           persist, exps, outsb, outtp, psbig, pssmall, ufd)

            for _ in range(rep):
                body()

    nc.compile()
    return nc


def _emit_body(nc, tc, mybir, s, qt, kt, xt, wq, wk, wv, bq_t, bk_t, bvb_t,
               id_t, out, persist, exps, outsb, outtp, psbig, pssmall, ufd=2):
    f32 = mybir.dt.float32
    bf16 = mybir.dt.bfloat16
    Exp = mybir.ActivationFunctionType.Exp
    qg_n = s // QW
    kc_n = s // 128
    sb_n = s // QW
    if True:
        if True:
            def proj_qk(dst, w, b_t, sb):
                ps = psbig.tile([128, QW], f32, name="ps_proj", tag="sc")
                for c in range(4):
                    nc.tensor.matmul(
                        ps[:],
                        lhsT=w[c][:],
                        rhs=xt[c][:, sb * QW:(sb + 1) * QW],
                        start=(c == 0),
                        stop=(c == 3),
                    )
                nc.vector.tensor_scalar_add(
                    dst[:, sb * QW:(sb + 1) * QW], ps[:], b_t[:]
                )

            # K fully, then Q block 0 (enough to start attention qg 0)
            for sb in range(sb_n):
                proj_qk(kt, wk, bk_t, sb)
            proj_qk(qt, wq, bq_t, 0)

            # ---- V projection directly in [s, o] layout ----
            v_sb = [persist.tile([128, kc_n * 65], bf16, name=f"vsb{h}",
                                 tag=f"vsb{h}") for h in (0, 1)]
            for h in (0, 1):
                nc.vector.memset(v_sb[h][:], 1.0)
            for sb in range(kc_n):
                ps = pssmall.tile([128, 128], f32, name="ps_vp", tag="av")
                for c in range(4):
                    nc.tensor.matmul(
                        ps[:],
                        lhsT=xt[c][:, sb * 128:(sb + 1) * 128],
                        rhs=wv[c][:],
                        start=(c == 0),
                        stop=(c == 3),
                    )
                for h in (0, 1):
                    nc.vector.tensor_add(
                        v_sb[h][:, sb * 65:sb * 65 + 64],
                        ps[:, h * 64:(h + 1) * 64],
                        bvb_t[:, h * 64:(h + 1) * 64],
                    )

            # ---- attention ----
            for qg in range(qg_n):
                if qg + 1 < qg_n:
                    proj_qk(qt, wq, bq_t, qg + 1)
                av = [pssmall.tile([65, QW], f32, name="av", tag="av")
                      for _ in (0, 1)]
                units = [(kc, h) for kc in range(kc_n) for h in (0, 1)]
                for u0 in range(0, len(units), ufd):
                    grp = units[u0:u0 + ufd]
                    ps = psbig.tile([128, len(grp) * QW], f32, name="ps_sc",
                                    tag="sc")
                    for i, (kc, h) in enumerate(grp):
                        nc.tensor.matmul(
                            ps[:, i * QW:(i + 1) * QW],
                            lhsT=kt[h * HD:(h + 1) * HD,
                                    kc * 128:(kc + 1) * 128],
                            rhs=qt[h * HD:(h + 1) * HD,
                                   qg * QW:(qg + 1) * QW],
                            start=True,
                            stop=True,
                        )
                    ex = exps.tile([128, len(grp) * QW], bf16, name="ex",
                                   tag="exp")
                    nc.scalar.activation(ex[:], ps[:], Exp)
                    for i, (kc, h) in enumerate(grp):
                        nc.tensor.matmul(
                            av[h][:],
                            lhsT=v_sb[h][:, kc * 65:kc * 65 + 65],
                            rhs=ex[:, i * QW:(i + 1) * QW],
                            start=(kc == 0),
                            stop=(kc == kc_n - 1),
                        )
                # tail: transpose + normalize + store
                ot = [outsb.tile([128, OD], f32, name="ot", tag="outsb")
                      for _ in range(4)]
                for h in (0, 1):
                    outt = outtp.tile([65, QW], f32, name="outt", tag="outt")
                    nc.vector.tensor_copy(outt[:], av[h][:])
                    for blk in range(4):
                        tp = psbig.tile([128, 65], f32, name="tp", tag="sc")
                        nc.tensor.transpose(
                            tp[:],
                            outt[:, blk * 128:(blk + 1) * 128],
                            id_t[0:65, 0:65],
                        )
                        rs = outsb.tile([128, 1], f32, name="rs", tag="rs")
                        nc.vector.reciprocal(rs[:], tp[:, 64:65])
                        nc.vector.tensor_scalar_mul(
                            ot[blk][:, h * HD:(h + 1) * HD], tp[:, 0:64], rs[:]
                        )
                for blk in range(4):
                    r0 = qg * QW + blk * 128
                    nc.sync.dma_start(out[r0:r0 + 128, :], ot[blk][:])


def _get_nc(s=S):
    if s not in _CACHE:
        _CACHE[s] = _build(s)
    return _CACHE[s]


def _shard_inputs(x, Wq, bq, Wk, bk, Wv, bv):
    import ml_dtypes

    bf16 = ml_dtypes.bfloat16
    f32 = np.float32
    ident = np.eye(128, dtype=f32)
    # batch-level tensors shared by 4 cores each; weight slices by 2 cores.
    xTb = [np.ascontiguousarray(x[b].T).astype(bf16) for b in range(B)]
    wq_s, wk_s, wv_s, bq_s, bk_s, bvb_s = [], [], [], [], [], []
    for hp in range(4):
        r = slice(128 * hp, 128 * hp + 128)
        wq_s.append(np.ascontiguousarray((Wq[r] * 0.125).T).astype(bf16))
        wk_s.append(np.ascontiguousarray(Wk[r].T).astype(bf16))
        wv_s.append(np.ascontiguousarray(Wv[r].T).astype(bf16))
        bq_s.append((bq[r] * 0.125).reshape(128, 1).astype(f32))
        bk_s.append(bk[r].reshape(128, 1).astype(f32))
        bvb_s.append(np.tile(bv[r][None, :], (128, 1)).astype(f32))
    in_maps = []
    for c in range(8):
        b, hp = divmod(c, 4)
        in_maps.append({
            "xT": xTb[b],
            "wqT": wq_s[hp],
            "wkT": wk_s[hp],
            "wvT": wv_s[hp],
            "bq": bq_s[hp],
            "bk": bk_s[hp],
            "bvb": bvb_s[hp],
            "ident": ident,
        })
    return in_maps


def kernel(x, Wq, bq, Wk, bk, Wv, bv, _trace=False):
    from concourse.bass_utils import run_bass_kernel_spmd

    x = np.asarray(x, dtype=np.float32)
    Wq = np.asarray(Wq, dtype=np.float32)
    bq = np.asarray(bq, dtype=np.float32)
    Wk = np.asarray(Wk, dtype=np.float32)
    bk = np.asarray(bk, dtype=np.float32)
    Wv = np.asarray(Wv, dtype=np.float32)
    bv = np.asarray(bv, dtype=np.float32)

    nc = _get_nc(S)
    in_maps = _shard_inputs(x, Wq, bq, Wk, bk, Wv, bv)
    res = run_bass_kernel_spmd(nc, in_maps, core_ids=list(range(8)), trace=_trace)
    kernel._last_results = res

    out = np.empty((B, S, D), dtype=np.float32)
    for c in range(8):
        b, hp = divmod(c, 4)
        out[b, :, 128 * hp:128 * hp + 128] = res.results[c]["out"]
    return out
